# revision 13
# baseline (speedup 1.0000x reference)
"""BiLSTM-CRF loss on 8 Trainium2 NeuronCores (Bass/Tile kernel).

Strategy: time-sharding. T=1024 is split into 8 chunks of 128 steps; each core
runs both LSTM directions over its chunk plus a warmup window (LSTM state and
the CRF forward recursion both forget their initialization geometrically, so a
48-step LSTM warmup / 8-step CRF warmup reproduces the exact fp32 state; the
boundary cores pin exact initial state via mask inputs instead).
Each core computes its chunk's emissions, the CRF partition increments (exp
domain with a log(L) shift; per-core log-mass growth telescopes into the full
denominator), and the gold-path score terms (one-hot matmuls; start/end terms
folded in through solve(trans, .) encodings of the label stream). The host
only slices/pads inputs per core and sums per-core partial outputs.

Device numerics: bf16 storage for h/tanh outputs and matmul operands, fp32
PSUM accumulation. Sigmoid is expressed via tanh(x/2) (weights pre-scaled on
the host) so ONE activation op covers all four gates. The embedding lookup and
input projection are fused into a [4H, V] table gathered by token id with the
GPSIMD ap_gather instruction and added to PSUM via an identity matmul.
"""
import numpy as np
import ml_dtypes

BF16 = ml_dtypes.bfloat16

# model dims
V, E, HD, L = 256, 128, 256, 15
B, T = 128, 1024
H = HD // 2
NCORE = 8
CH = T // NCORE

# warmups (validated numerically: LSTM err 6.7e-7 @ 48; CRF err 3e-15 @ 8)
WL = 48
WC = 8
S = WL + WC + 129          # LSTM slots per direction
SE = WC + 129              # emission/CRF steps
MU = float(np.log(L))      # CRF exp-domain shift
CHG = 8                    # gather chunk size (slots)

DEBUG = False


def _bf(a):
    return np.asarray(a, np.float32).astype(BF16)


def _f32(a):
    return np.asarray(a, np.float32)


def _wrap_idx(ids_flat):
    """ap_gather index layout: index for output position n lives at
    partition 16*g + (n % 16), column n // 16 (same list in every group g)."""
    n = ids_flat.shape[0]
    assert n % 16 == 0
    cols = ids_flat.reshape(n // 16, 16).T.astype(np.int16)
    return np.tile(cols, (8, 1))


def prep_inputs(inputs):
    """Build the 8 per-core in_maps (numpy slicing / weight preprocessing)."""
    ids = np.asarray(inputs["input_ids"]).astype(np.int64)
    labels = np.asarray(inputs["labels"]).astype(np.int64)
    emb = _f32(inputs["emb_table"]).copy()
    emb[0] = 0.0
    trans = _f32(inputs["trans"])
    start = _f32(inputs["start_trans"])
    end = _f32(inputs["end_trans"])
    fc_w = _f32(inputs["fc_w"])
    fc_b = _f32(inputs["fc_b"])

    rowscale = np.ones((4 * H, 1), np.float32)
    rowscale[2 * H:3 * H] = 2.0            # g-gate rows x2 (tanh trick)

    com = {}
    for d, (wihn, whhn, bn) in (
        ("f", ("w_ih_f", "w_hh_f", "b_f")),
        ("b", ("w_ih_b", "w_hh_b", "b_b")),
    ):
        wih = _f32(inputs[wihn]); whh = _f32(inputs[whhn]); bias = _f32(inputs[bn])
        F = rowscale * (wih @ emb.T + bias[:, None])          # [512, V]
        # xp matmul weights: lhsT tiles [128 vocab-half, 128 gate-chunk],
        # packed (g, half)-major along the free dim
        blocks = [F[g * H:(g + 1) * H, hv * H:(hv + 1) * H].T
                  for g in range(4) for hv in range(2)]
        com[f"xpw{d}"] = _bf(np.concatenate(blocks, 1))       # [128, 1024]
        whh_s = rowscale * whh * 0.5                          # x0.5: h stored as 2h
        com[f"whh{d}"] = _bf(
            np.concatenate([whh_s[g * H:(g + 1) * H].T for g in range(4)], 1))
    com["io0"] = np.arange(128, dtype=np.float32).reshape(128, 1)
    com["io1"] = np.arange(128, 256, dtype=np.float32).reshape(128, 1)
    com["fcwf"] = _bf(0.5 * fc_w[:, :H].T)                    # [128, 15]
    com["fcwb"] = _bf(0.5 * fc_w[:, H:].T)
    com["fcb_exp"] = (fc_b - MU).reshape(L, 1).astype(np.float32)
    com["fcb_num"] = fc_b.reshape(L, 1).astype(np.float32)
    com["etr"] = np.exp(trans).astype(np.float32)
    com["trl"] = _bf(trans)
    com["estart"] = np.exp(start).reshape(L, 1).astype(np.float32)
    com["eend"] = np.exp(end).reshape(L, 1).astype(np.float32)
    com["ones15"] = np.ones((L, 1), np.float32)

    v_enc = np.linalg.solve(trans.T.astype(np.float64), start.astype(np.float64))
    w_enc = np.linalg.solve(trans.astype(np.float64), end.astype(np.float64))

    in_maps = []
    for k in range(NCORE):
        t0 = k * CH
        m = dict(com)
        tf = t0 - WC - WL + np.arange(S)
        tb = t0 + 128 + WL - np.arange(S)
        for d, tt in (("f", tf), ("b", tb)):
            tcl = np.clip(tt, 0, T - 1)
            idsw = np.where(((tt >= 0) & (tt < T))[:, None], ids[:, tcl].T, 0)  # [S,B]
            m[f"ids{d}"] = np.broadcast_to(
                idsw.reshape(1, S * B), (128, S * B)).astype(np.uint8).copy()
        oh = np.zeros((L, 130, B), np.float32)
        for q in range(130):
            tq = t0 - 1 + q
            if 0 <= tq < T:
                oh[labels[:, tq], q, np.arange(B)] = 1.0
        if k == 0:
            oh[:, 0, :] = v_enc[:, None]
        oh[:, 129, :] = w_enc[:, None] if k == NCORE - 1 else 0.0
        m["oh"] = _bf(oh.reshape(L, 130 * B))
        m["mskf"] = np.full((H, 1), 0.0 if k == 0 else 1.0, np.float32)
        m["mskb"] = np.full((H, 1), 0.0 if k == NCORE - 1 else 1.0, np.float32)
        m["cm"] = np.full((L, 1), 0.0 if k == 0 else 1.0, np.float32)
        m["icm"] = np.full((L, 1), 1.0 if k == 0 else 0.0, np.float32)
        in_maps.append(m)
    return in_maps


def assemble(outs):
    """Combine the per-core [8,128] result tensors into the scalar loss."""
    res = [np.asarray(o, np.float64) for o in outs]
    denom = res[0][0] + MU
    for k in range(NCORE - 1):
        denom = denom + (res[k][2] - res[k][0]) + 128 * MU
    denom = denom + (res[7][1] - res[7][0]) + 127 * MU
    denom = denom + (res[7][3] - res[7][1])
    score = sum(r[4] for r in res)
    return np.float32(-(score - denom).mean())


# ---------------------------------------------------------------------------
# numpy emulation of the device program (debug reference for bring-up)
# ---------------------------------------------------------------------------
def sim_core(m):
    bf = lambda a: a.astype(BF16).astype(np.float32)
    out = np.zeros((8, B), np.float32)
    tabs = {}
    for d in "fb":
        w = _f32(m[f"xpw{d}"])                   # [128, 8*128]
        F = np.zeros((4 * H, V), np.float32)
        for g in range(4):
            for hv in range(2):
                blk = w[:, (g * 2 + hv) * H:(g * 2 + hv + 1) * H]   # [v, gate]
                F[g * H:(g + 1) * H, hv * H:(hv + 1) * H] = blk.T
        tabs[d] = F
    whh = {d: _f32(m[f"whh{d}"]) for d in "fb"}
    idsd = {d: m[f"ids{d}"][0].reshape(S, B).astype(np.int64) for d in "fb"}

    hist = {}
    for d, mk, rs in (("f", m["mskf"], WL + WC), ("b", m["mskb"], WL + 1)):
        h2 = np.zeros((H, B), np.float32)
        C = np.zeros((H, B), np.float32)
        hh = np.zeros((S, H, B), np.float32)
        for s in range(S):
            if s == rs:
                h2 = bf(h2 * mk)
                C = bf(C * mk)
            xp = bf(tabs[d][:, idsd[d][s]])
            psum = xp.copy()
            for g in range(4):
                lhsT = whh[d][:, g * H:(g + 1) * H]
                psum[g * H:(g + 1) * H] += lhsT.T @ h2
            Tt = bf(np.tanh(0.5 * psum))
            Ti, Tf, Tg, To = Tt.reshape(4, H, B)
            a = bf((Tf + 1.0) * C)
            b2 = bf((Ti + 1.0) * Tg)
            C = bf(0.5 * a + b2)
            tc = bf(np.tanh(0.5 * C))
            h2 = bf((To + 1.0) * tc)
            hh[s] = h2
        hist[d] = hh

    fcwf = _f32(m["fcwf"]); fcwb = _f32(m["fcwb"])
    oh = _f32(m["oh"]).reshape(L, 130, B)
    expem = np.zeros((SE, L, B), np.float32)
    num_acc = np.zeros((L, B), np.float32)
    for e0 in range(0, SE, 4):
        cnt = min(4, SE - e0)
        psum_em = np.zeros((cnt, L, B), np.float32)
        for j in range(cnt):
            e = e0 + j
            psum_em[j] = fcwf.T @ hist["f"][e + WL] + fcwb.T @ hist["b"][128 + WC + WL - e]
        expem[e0:e0 + cnt] = bf(np.exp(psum_em + _f32(m["fcb_exp"])[None]))
        for j in range(cnt):
            e = e0 + j
            if WC <= e < WC + 128:
                num_acc += bf((psum_em[j] + _f32(m["fcb_num"])) * oh[:, e - WC + 1])
            if WC <= e <= WC + 128:
                trg = _f32(m["trl"]).T @ oh[:, e - WC]
                num_acc += bf(trg * oh[:, e - WC + 1])
    etr = _f32(m["etr"])
    A = None
    for e in range(SE):
        if e == 0:
            A = expem[0].copy()
        else:
            A = (etr.T @ A) * expem[e]
        if e == WC:
            A = A * m["cm"] + (m["estart"] * m["icm"]) * expem[e]
            out[0] = np.log(A.sum(0))
        if e == WC + 127:
            out[1] = np.log(A.sum(0))
            out[3] = np.log((A * m["eend"]).sum(0))
        if e == WC + 128:
            out[2] = np.log(A.sum(0))
    out[4] = num_acc.sum(0)
    return out


# ---------------------------------------------------------------------------
# Bass/Tile device program
# ---------------------------------------------------------------------------
_CACHE = {}


def build_nc():
    import contextlib
    import concourse.bass as bass
    import concourse.tile as tile
    from concourse import bacc, mybir
    fp32 = mybir.dt.float32
    bf16 = mybir.dt.bfloat16
    i16 = mybir.dt.int16
    AF = mybir.ActivationFunctionType
    AL = mybir.AluOpType

    nc = bacc.Bacc("TRN2", target_bir_lowering=False, debug=False)
    dram = {}

    def din(name, shape, dt=fp32):
        dram[name] = nc.dram_tensor(name, shape, dt, kind="ExternalInput")[:]

    u8 = mybir.dt.uint8
    for d in "fb":
        din(f"xpw{d}", [H, 8 * H], bf16)
        din(f"whh{d}", [H, 4 * H], bf16)
        din(f"ids{d}", [128, S * B], u8)
    din("io0", [128, 1])
    din("io1", [128, 1])
    din("fcwf", [H, L], bf16); din("fcwb", [H, L], bf16)
    din("fcb_exp", [L, 1]); din("fcb_num", [L, 1])
    din("etr", [L, L]); din("trl", [L, L], bf16)
    din("estart", [L, 1]); din("eend", [L, 1]); din("ones15", [L, 1])
    din("oh", [L, 130 * B], bf16)
    din("mskf", [H, 1]); din("mskb", [H, 1])
    din("cm", [L, 1]); din("icm", [L, 1])
    res_d = nc.dram_tensor("res", [8, B], fp32, kind="ExternalOutput")[:]
    if DEBUG:
        dbg_d = nc.dram_tensor("dbg", [128, 4 * B], fp32, kind="ExternalOutput")[:]

    with tile.TileContext(nc) as tc:
        with contextlib.ExitStack() as ctx:
            cpool = ctx.enter_context(tc.tile_pool(name="const", bufs=1))
            hpool = ctx.enter_context(tc.tile_pool(name="hist", bufs=1))
            xpool = ctx.enter_context(tc.tile_pool(name="xp", bufs=2))
            wpool = ctx.enter_context(tc.tile_pool(name="work", bufs=3))
            apool = ctx.enter_context(tc.tile_pool(name="crf", bufs=2))

            def load(name, shape, dt=fp32):
                t = cpool.tile(shape, dt, tag=name, name=name)
                nc.sync.dma_start(t[:], dram[name])
                return t

            u8 = mybir.dt.uint8
            xpw = {d: load(f"xpw{d}", [H, 8 * H], bf16) for d in "fb"}
            whh = {d: load(f"whh{d}", [H, 4 * H], bf16) for d in "fb"}
            io = [load("io0", [128, 1]), load("io1", [128, 1])]
            fcw = {"f": load("fcwf", [H, L], bf16), "b": load("fcwb", [H, L], bf16)}
            fcb_exp = load("fcb_exp", [L, 1]); fcb_num = load("fcb_num", [L, 1])
            etr = load("etr", [L, L]); trl = load("trl", [L, L], bf16)
            estart = load("estart", [L, 1]); eend = load("eend", [L, 1])
            ones15 = load("ones15", [L, 1])
            msk = {"f": load("mskf", [H, 1]), "b": load("mskb", [H, 1])}
            cm = load("cm", [L, 1]); icm = load("icm", [L, 1])

            # persistent state: h history (slot s at cols (s+1)*B; col 0:B = zero
            # init), C state, expem, accumulators
            hist = {d: hpool.tile([H, (S + 1) * B], bf16, tag=f"hist{d}", name=f"hist{d}") for d in "fb"}
            Cst = {d: hpool.tile([H, B], bf16, tag=f"C{d}", name=f"C{d}") for d in "fb"}
            for d in "fb":
                nc.vector.memset(hist[d][:, 0:B], 0.0)
                nc.vector.memset(Cst[d][:], 0.0)
            numacc = hpool.tile([L, B], fp32, tag="numacc", name="numacc")
            nc.vector.memset(numacc[:], 0.0)
            rows = {r: hpool.tile([1, B], fp32, tag=f"row{r}", name=f"row{r}")
                    for r in range(5)}

            # ---------------- phase B: the two LSTM chains ----------------
            pbctx = ctx.enter_context(contextlib.ExitStack())
            ppool = pbctx.enter_context(tc.tile_pool(name="ps", bufs=4, space="PSUM"))
            RESET = {"f": WL + WC, "b": WL + 1}
            oh_bufs = {}

            def get_oh(d, c):
                # one-hot vocab-half tiles for a CHG-slot chunk, built by DVE
                if (d, c) not in oh_bufs:
                    s0 = c * CHG
                    cnt = min(CHG, S - s0)
                    idch = xpool.tile([128, CHG * B], u8, tag=f"ic{d}", name=f"ic{d}")
                    nc.sync.dma_start(idch[:, :cnt * B],
                                      dram[f"ids{d}"][:, s0 * B:(s0 + cnt) * B])
                    bufs = []
                    for hv in (0, 1):
                        bb = xpool.tile([128, CHG * B], bf16, tag=f"oh{d}{hv}", name=f"oh{d}{hv}")
                        nc.vector.tensor_scalar(
                            bb[:, :cnt * B], idch[:, :cnt * B],
                            io[hv][:], None, AL.is_equal)
                        bufs.append(bb)
                    oh_bufs[(d, c)] = bufs
                return oh_bufs[(d, c)]

            for s in range(S):
                for d in "fb":
                    if s == RESET[d]:
                        hs_prev = hist[d][:, s * B:(s + 1) * B]
                        nc.vector.tensor_scalar(hs_prev, hs_prev, msk[d][:], None, AL.mult)
                        nc.vector.tensor_scalar(Cst[d][:], Cst[d][:], msk[d][:], None, AL.mult)
                    c = s // CHG
                    j = s - c * CHG
                    ohb = get_oh(d, c)
                    if j == 0 and c + 1 < (S + CHG - 1) // CHG:
                        get_oh(d, c + 1)
                    ps = ppool.tile([H, 4 * B], fp32, tag=f"ps{d}", name=f"ps{d}")
                    h_prev = hist[d][:, s * B:(s + 1) * B]
                    for g in range(4):
                        for hv in (0, 1):
                            nc.tensor.matmul(
                                ps[:, g * B:(g + 1) * B],
                                xpw[d][:, (g * 2 + hv) * H:(g * 2 + hv + 1) * H],
                                ohb[hv][:, j * B:(j + 1) * B],
                                start=(g == 0 and hv == 0), stop=False)
                    for g in range(4):
                        nc.tensor.matmul(
                            ps[:, g * B:(g + 1) * B],
                            whh[d][:, g * H:(g + 1) * H], h_prev,
                            start=False, stop=(g == 3))
                    Tt = wpool.tile([H, 4 * B], bf16, tag=f"T{d}", name=f"T{d}")
                    nc.scalar.activation(Tt[:], ps[:], AF.Tanh, scale=0.5)
                    Ti = Tt[:, 0:B]; Tf = Tt[:, B:2 * B]
                    Tg = Tt[:, 2 * B:3 * B]; To = Tt[:, 3 * B:4 * B]
                    aa = wpool.tile([H, B], bf16, tag=f"a{d}", name=f"a{d}")
                    nc.vector.scalar_tensor_tensor(aa[:], Tf, 1.0, Cst[d][:], AL.add, AL.mult)
                    bb2 = wpool.tile([H, B], bf16, tag=f"b{d}", name=f"b{d}")
                    nc.vector.scalar_tensor_tensor(bb2[:], Ti, 1.0, Tg, AL.add, AL.mult)
                    nc.vector.scalar_tensor_tensor(Cst[d][:], aa[:], 0.5, bb2[:], AL.mult, AL.add)
                    tcl = wpool.tile([H, B], bf16, tag=f"tc{d}", name=f"tc{d}")
                    nc.scalar.activation(tcl[:], Cst[d][:], AF.Tanh, scale=0.5)
                    h_new = hist[d][:, (s + 1) * B:(s + 2) * B]
                    nc.vector.scalar_tensor_tensor(h_new, To, 1.0, tcl[:], AL.add, AL.mult)

            # ---------------- phase C: emissions + CRF + numerator --------
            pbctx.close()
            p2pool = ctx.enter_context(tc.tile_pool(name="ps2", bufs=2, space="PSUM"))
            p1pool = ctx.enter_context(tc.tile_pool(name="ps1", bufs=2, space="PSUM"))
            Acur = None
            for e0 in range(0, SE, 4):
                cnt = min(4, SE - e0)
                pse = p2pool.tile([L, 4 * B], fp32, tag="psem", name="psem")
                sf = e0 + WL + 1
                nc.tensor.matmul(pse[:, :cnt * B], fcw["f"][:],
                                 hist["f"][:, sf * B:(sf + cnt) * B],
                                 start=True, stop=False)
                for j in range(cnt):
                    sb = 129 + WC + WL - (e0 + j)
                    nc.tensor.matmul(pse[:, j * B:(j + 1) * B], fcw["b"][:],
                                     hist["b"][:, sb * B:(sb + 1) * B],
                                     start=False, stop=(j == cnt - 1))
                exch = xpool.tile([L, 4 * B], bf16, tag="exch", name="exch")
                nc.scalar.activation(exch[:, :cnt * B], pse[:, :cnt * B],
                                     AF.Exp, bias=fcb_exp[:])
                # numerator: emission part (t in [t0, t0+128)) and trans part
                lo = max(e0, WC)
                hi_em = min(e0 + cnt, WC + 128)
                hi_tr = min(e0 + cnt, WC + 129)
                if lo < hi_tr:
                    qa = lo - WC
                    qn = hi_tr - WC + 1 - qa        # prev..cur window, <=5 slots
                    ohch = xpool.tile([L, 5 * B], bf16, tag="ohch", name="ohch")
                    nc.sync.dma_start(
                        ohch[:, :qn * B],
                        dram["oh"][:, qa * B:(qa + qn) * B])
                if lo < hi_em:
                    n = hi_em - lo
                    scr = wpool.tile([L, 4 * B], bf16, tag="scr", name="scr")
                    nc.vector.scalar_tensor_tensor(
                        scr[:, :n * B], pse[:, (lo - e0) * B:(hi_em - e0) * B],
                        fcb_num[:], ohch[:, B:(n + 1) * B],
                        AL.add, AL.mult)
                    red = wpool.tile([L, B], fp32, tag="red", name="red")
                    nc.vector.tensor_reduce(
                        red[:], scr[:, :n * B].rearrange("p (q b) -> p b q", b=B),
                        mybir.AxisListType.X, AL.add)
                    nc.vector.tensor_tensor(numacc[:], numacc[:], red[:], AL.add)
                if lo < hi_tr:
                    n = hi_tr - lo
                    pst = p2pool.tile([L, 4 * B], fp32, tag="pstr", name="pstr")
                    nc.tensor.matmul(pst[:, :n * B], trl[:],
                                     ohch[:, 0:n * B],
                                     start=True, stop=True)
                    scr2 = wpool.tile([L, 4 * B], bf16, tag="scr2", name="scr2")
                    nc.vector.tensor_tensor(
                        scr2[:, :n * B], pst[:, :n * B],
                        ohch[:, B:(n + 1) * B],
                        AL.mult)
                    red2 = wpool.tile([L, B], fp32, tag="red2", name="red2")
                    nc.vector.tensor_reduce(
                        red2[:], scr2[:, :n * B].rearrange("p (q b) -> p b q", b=B),
                        mybir.AxisListType.X, AL.add)
                    nc.vector.tensor_tensor(numacc[:], numacc[:], red2[:], AL.add)
                # CRF steps for this chunk
                for j in range(cnt):
                    e = e0 + j
                    eslice = exch[:, j * B:(j + 1) * B]
                    if e == 0:
                        Acur = apool.tile([L, B], fp32, tag="A", name="A")
                        nc.vector.tensor_copy(Acur[:], eslice)
                    else:
                        psA = p1pool.tile([L, B], fp32, tag="psA", name="psA")
                        nc.tensor.matmul(psA[:], etr[:], Acur[:], start=True, stop=True)
                        Anew = apool.tile([L, B], fp32, tag="A", name="A")
                        nc.vector.tensor_tensor(Anew[:], psA[:], eslice, AL.mult)
                        Acur = Anew
                    if e == WC:
                        t3 = wpool.tile([L, B], fp32, tag="t3", name="t3")
                        nc.vector.tensor_scalar(t3[:], eslice, estart[:], icm[:],
                                                AL.mult, AL.mult)
                        Ab = apool.tile([L, B], fp32, tag="A", name="A")
                        nc.vector.scalar_tensor_tensor(Ab[:], Acur[:], cm[:], t3[:],
                                                       AL.mult, AL.add)
                        Acur = Ab
                    if e in (WC, WC + 127, WC + 128):
                        psL = p1pool.tile([1, B], fp32, tag="psL", name="psL")
                        nc.tensor.matmul(psL[:], ones15[:], Acur[:], start=True, stop=True)
                        row = {WC: 0, WC + 127: 1, WC + 128: 2}[e]
                        nc.scalar.activation(rows[row][:], psL[:], AF.Ln)
                        if e == WC + 127:
                            psW = p1pool.tile([1, B], fp32, tag="psL", name="psL")
                            nc.tensor.matmul(psW[:], eend[:], Acur[:], start=True, stop=True)
                            nc.scalar.activation(rows[3][:], psW[:], AF.Ln)

            psN = p1pool.tile([1, B], fp32, tag="psL", name="psL")
            nc.tensor.matmul(psN[:], ones15[:], numacc[:], start=True, stop=True)
            nc.vector.tensor_copy(rows[4][:], psN[:])
            for r in range(5):
                nc.sync.dma_start(res_d[r:r + 1, :], rows[r][:])
            if DEBUG:
                dwork = wpool.tile([128, 4 * B], fp32, tag="dbg", name="dbg")
                nc.vector.tensor_copy(dwork[:, 0:B], hist["f"][:, (WL + WC + 1) * B:(WL + WC + 2) * B])
                nc.vector.tensor_copy(dwork[:, B:2 * B], hist["b"][:, (WL + 2) * B:(WL + 3) * B])
                nc.vector.tensor_copy(dwork[:L, 3 * B:4 * B], numacc[:])
                nc.sync.dma_start(dbg_d, dwork[:])

    nc.compile()
    return nc


def _make_runner():
    """Compile the bass program once into a reusable 8-core PJRT callable."""
    import jax
    import numpy as np
    from jax.sharding import Mesh, PartitionSpec
    from jax.experimental.shard_map import shard_map
    from concourse import bass2jax, mybir
    bass2jax.install_neuronx_cc_hook()

    nc = build_nc()
    pname = nc.partition_id_tensor.name if nc.partition_id_tensor else None
    in_names, out_names, out_avals, zero_outs = [], [], [], []
    for alloc in nc.m.functions[0].allocations:
        if not isinstance(alloc, mybir.MemoryLocationSet):
            continue
        name = alloc.memorylocations[0].name
        if alloc.kind == "ExternalInput":
            if name != pname:
                in_names.append(name)
        elif alloc.kind == "ExternalOutput":
            shape = tuple(alloc.tensor_shape)
            dt = mybir.dt.np(alloc.dtype)
            out_names.append(name)
            out_avals.append(jax.core.ShapedArray(shape, dt))
            zero_outs.append(np.zeros(shape, dt))
    n_params = len(in_names)
    all_names = in_names + out_names
    if pname is not None:
        all_names = all_names + [pname]

    def _body(*args):
        operands = list(args)
        if pname is not None:
            operands.append(bass2jax.partition_id_tensor())
        outs = bass2jax._bass_exec_p.bind(
            *operands, out_avals=tuple(out_avals), in_names=tuple(all_names),
            out_names=tuple(out_names), lowering_input_output_aliases=(),
            sim_require_finite=True, sim_require_nnan=True, nc=nc)
        return tuple(outs)

    devices = jax.devices()[:NCORE]
    mesh = Mesh(np.asarray(devices), ("core",))
    donate = tuple(range(n_params, n_params + len(out_names)))
    fn = jax.jit(
        shard_map(_body, mesh=mesh,
                  in_specs=(PartitionSpec("core"),) * (n_params + len(out_names)),
                  out_specs=(PartitionSpec("core"),) * len(out_names),
                  check_rep=False),
        donate_argnums=donate, keep_unused=True)

    def prepare(in_maps):
        return [np.concatenate([np.asarray(m[n]) for m in in_maps], 0)
                for n in in_names]

    def run_prepared(concat_in):
        concat_zero = [np.zeros((NCORE * z.shape[0], *z.shape[1:]), z.dtype)
                       for z in zero_outs]
        return fn(*concat_in, *concat_zero)

    def run(in_maps):
        arrs = run_prepared(prepare(in_maps))
        return [{name: np.asarray(arrs[i]).reshape(NCORE, *out_avals[i].shape)[c]
                 for i, name in enumerate(out_names)}
                for c in range(NCORE)]

    run.prepare = prepare
    run.run_prepared = run_prepared
    return run


def _run_device(in_maps):
    if "run" not in _CACHE:
        _CACHE["run"] = _make_runner()
    return _CACHE["run"](in_maps)


def kernel(**inputs):
    in_maps = prep_inputs(inputs)
    outs = _run_device(in_maps)
    return assemble([o["res"] for o in outs])
